# revision 1
# baseline (speedup 1.0000x reference)
"""AttentionGNN (3-layer TransformerConv) Trainium2 kernel.

  - Nodes partitioned across 8 cores by dst range (12500 each); edges routed to
    the core owning their destination.
  - Math restructure: scores = <q~[dst], x[src]> with q~ = (x Wq + bq) Wk^T/sqrt(C)
    (the bk term is a per-dst softmax constant -> cancels; segment-max dropped --
    scores are in [-2, 2.1]).  out = (sum w x[src]) / (sum w) @ Wv + (bv+bs) + x Ws.
  - Edge phase: slots sorted by (src quarter, dst).  Per chunk of Mc*128 slots:
    dma_gather x[src] rows (int16 idx local to the 25k-row quarter) and q~[dst]
    rows from a per-core q~ table, DVE mul + grouped-reduce -> scores, ACT exp,
    payload (w*x || w), then dma_scatter_add accumulates each slot's payload row
    into aggd[dst] in DRAM (CCE add).  All pads point at dump rows.
  - Dense phases on PE: q~ = x A (ones-row augmented) and
    out = (agg/denom) Wv + x Ws_aug, ReLU fused on ACT.
  - Host mediates inter-layer exchange (3 SPMD launches).
"""

import math
import os

import numpy as np

N_NODES = 100000
N_EDGES = 1600000
NCORES = 8
NL = N_NODES // NCORES          # 12500
P = 128
J = (NL + P - 1) // P           # 98
NJ = P * J                      # 12544
NRA = NJ + P                    # aggd rows (12672), last row = dump
QCH = 4                         # src quarters
QSZ = N_NODES // QCH            # 25000
MC = 64                         # slot columns per chunk
NSUB = 8                        # sub-calls per chunk: 1024-idx SWDGE calls (HW carveout limit)
CD = 64                         # unified feature width (layer0 zero-padded)

_PLAN_CACHE = {}


def _wrap_idx(lst, ncols):
    """int16 list -> [128, ncols] wrapped-in-16 + replicated-across-groups."""
    n = len(lst)
    out = np.zeros((P, ncols), np.int16)
    cols = (n + 15) // 16
    pad = np.zeros(cols * 16 - n, np.int16)
    w = np.concatenate([lst.astype(np.int16), pad]).reshape(cols, 16).T
    for g in range(8):
        out[g * 16:(g + 1) * 16, :cols] = w
    return out


def _build_plan(edge_index):
    key = hash(edge_index.tobytes())
    if key in _PLAN_CACHE:
        return _PLAN_CACHE[key]

    src = np.ascontiguousarray(edge_index[0]).astype(np.int64)
    dst = np.ascontiguousarray(edge_index[1]).astype(np.int64)

    percore = []
    Mr = 0
    for c in range(NCORES):
        lo, hi = c * NL, (c + 1) * NL
        esel = np.where((dst >= lo) & (dst < hi))[0]
        csrc = src[esel]
        cdst = (dst[esel] - lo).astype(np.int64)
        order = np.lexsort((cdst, csrc // QSZ))
        csrc, cdst = csrc[order], cdst[order]
        q = csrc // QSZ
        cnt = np.bincount(q, minlength=QCH)
        Mr = max(Mr, int(np.ceil(cnt.max() / (P * MC))))
        dq = np.bincount(cdst * QCH + q, minlength=NL * QCH)
        Mr = max(Mr, int(np.ceil((dq.max() + 1) / NSUB)))
        percore.append((csrc, cdst, q, cnt))

    ncols_r = Mr * MC                 # columns per region
    M = QCH * ncols_r                 # total slot columns
    nslot_r = ncols_r * P

    # scatter sub-call buckets: each dma_scatter_add covers BSLOT slots and must
    # not contain duplicate dst indices (HW CCE read-modify-write races).
    BSLOT = P * MC // NSUB
    NB = nslot_r // BSLOT              # buckets per quarter region

    plan = dict(M=M, Mr=Mr, cores=[])
    for c in range(NCORES):
        csrc, cdst, q, cnt = percore[c]
        xi = np.zeros((P, 8 * M), np.int16)
        qi = np.zeros((P, 8 * M), np.int16)
        si = np.zeros((P, 8 * M), np.int16)
        for r in range(QCH):
            sel = q == r
            ls_all = (csrc[sel] - r * QSZ).astype(np.int64)
            ld_all = cdst[sel].astype(np.int64)
            n = len(ls_all)
            # occurrence index within (dst): edges sorted by dst already
            occ = np.arange(n) - np.searchsorted(ld_all, ld_all)
            assert occ.max() < NB, (occ.max(), NB)
            # rank dsts to spread load: use dst id (uniformly distributed)
            bucket = (ld_all + occ) % NB
            # fill buckets sequentially
            border = np.lexsort((ld_all, bucket))
            bcnt = np.bincount(bucket, minlength=NB)
            assert bcnt.max() <= BSLOT, (bcnt.max(), BSLOT)
            pos = np.empty(n, np.int64)
            off = 0
            starts = np.zeros(NB + 1, np.int64)
            np.cumsum(bcnt, out=starts[1:])
            within = np.arange(n) - starts[bucket[border]]
            pos[border] = bucket[border] * BSLOT + within
            lsf = np.zeros(nslot_r, np.int16)
            lqf = np.full(nslot_r, NJ, np.int16)
            lsc = np.full(nslot_r, NRA - 1, np.int16)
            lsf[pos] = ls_all.astype(np.int16)
            lqf[pos] = ld_all.astype(np.int16)
            lsc[pos] = ld_all.astype(np.int16)
            sl = slice(r * 8 * ncols_r, (r + 1) * 8 * ncols_r)
            xi[:, sl] = _wrap_idx(lsf, 8 * ncols_r)
            qi[:, sl] = _wrap_idx(lqf, 8 * ncols_r)
            si[:, sl] = _wrap_idx(lsc, 8 * ncols_r)
        plan["cores"].append(dict(xi=xi, qi=qi, si=si))
    _PLAN_CACHE[key] = plan
    return plan


def _fold_weights(inp, li):
    Wq, bq = np.float64(inp[f"Wq{li}"]), np.float64(inp[f"bq{li}"])
    Wk = np.float64(inp[f"Wk{li}"])
    Wv, bv = np.float64(inp[f"Wv{li}"]), np.float64(inp[f"bv{li}"])
    Ws, bs = np.float64(inp[f"Ws{li}"]), np.float64(inp[f"bs{li}"])
    C = Wq.shape[1]
    Cin = Wq.shape[0]
    A = Wq @ Wk.T / math.sqrt(C)
    a0 = bq @ Wk.T / math.sqrt(C)
    A_aug = np.zeros((CD + 1, CD), np.float32)
    A_aug[:Cin, :Cin] = A
    A_aug[CD, :Cin] = a0
    Cout = Wv.shape[1]
    Wvp = np.zeros((CD, Cout), np.float32)
    Wvp[:Cin] = Wv
    Ws_aug = np.zeros((CD + 1, Cout), np.float32)
    Ws_aug[:Cin] = Ws
    Ws_aug[CD] = bv + bs
    return A_aug, Wvp, Ws_aug


def _build_layer_program(Cout, M, relu):
    from contextlib import ExitStack

    import concourse.tile as tile
    from concourse import bacc, mybir
    from concourse.masks import make_identity

    f32 = mybir.dt.float32
    i16 = mybir.dt.int16

    nc = bacc.Bacc("TRN2", target_bir_lowering=False, debug=False,
                   num_devices=NCORES)

    xtab = nc.dram_tensor("xtab", [N_NODES, CD], f32, kind="ExternalInput").ap()
    xpt = nc.dram_tensor("xpt", [CD + 1, NJ], f32, kind="ExternalInput").ap()
    xid = nc.dram_tensor("xi", [P, 8 * M], i16, kind="ExternalInput").ap()
    qid = nc.dram_tensor("qi", [P, 8 * M], i16, kind="ExternalInput").ap()
    sid = nc.dram_tensor("si", [P, 8 * M], i16, kind="ExternalInput").ap()
    Aaug = nc.dram_tensor("Aaug", [CD + 1, CD], f32, kind="ExternalInput").ap()
    Wv = nc.dram_tensor("Wv", [CD, Cout], f32, kind="ExternalInput").ap()
    WsA = nc.dram_tensor("WsA", [CD + 1, Cout], f32, kind="ExternalInput").ap()

    qtab = nc.dram_tensor("qtab", [NJ + 1, CD], f32).ap()
    aggd = nc.dram_tensor("aggd", [NRA, 2 * CD], f32).ap()
    out = nc.dram_tensor("out", [NJ, Cout], f32, kind="ExternalOutput").ap()

    nchunk = M // MC
    chunks_per_r = M // MC // QCH
    Cp = CD + 1

    with tile.TileContext(nc) as tc, ExitStack() as ctx:
        consts = ctx.enter_context(tc.tile_pool(name="consts", bufs=1))
        Asb = consts.tile([CD + 1, CD], f32)
        nc.sync.dma_start(Asb[:], Aaug[:])
        Wvsb = consts.tile([CD, Cout], f32)
        nc.sync.dma_start(Wvsb[:], Wv[:])
        WsAsb = consts.tile([CD + 1, Cout], f32)
        nc.sync.dma_start(WsAsb[:], WsA[:])
        ident = consts.tile([P, P], f32)
        make_identity(nc, ident[:])

        # zero-init aggd; denom col = 1.0 (real rows accumulated by CCE +=)
        zp = ExitStack()
        zpool = zp.enter_context(tc.tile_pool(name="zeros", bufs=1))
        zcols = NRA * 2 * CD // P
        zt = zpool.tile([P, zcols], f32)
        nc.gpsimd.memset(zt[:], 0.0)
        aggflat = aggd.rearrange("(p r) c -> p (r c)", p=P)
        nc.sync.dma_start(aggflat[:], zt[:])
        ot = zpool.tile([1, NRA - NL], f32)
        nc.gpsimd.memset(ot[:], 1.0)
        nc.sync.dma_start(aggd[NL:NRA, CD:CD + 1].rearrange("r c -> c r"), ot[:])
        zq = zpool.tile([1, CD], f32)
        nc.gpsimd.memset(zq[:], 0.0)
        nc.sync.dma_start(qtab[NJ:NJ + 1, :], zq[:])
        zp.close()

        # phase A: q~ table
        pa = ExitStack()
        pa_x = pa.enter_context(tc.tile_pool(name="pa_x", bufs=3))
        pa_ps = pa.enter_context(tc.tile_pool(name="pa_ps", bufs=2, space="PSUM"))
        pa_q = pa.enter_context(tc.tile_pool(name="pa_q", bufs=3))
        for t in range(J):
            xt = pa_x.tile([CD + 1, P], f32, tag="xt")
            nc.sync.dma_start(xt[:], xpt[:, t * P:(t + 1) * P])
            psq = pa_ps.tile([P, CD], f32)
            nc.tensor.matmul(psq[:], lhsT=xt[:], rhs=Asb[:], start=True, stop=True)
            qsb = pa_q.tile([P, CD], f32)
            nc.vector.tensor_copy(qsb[:], psq[:])
            nc.sync.dma_start(qtab[t * P:(t + 1) * P, :], qsb[:])
        pa.close()

        # phase B: edge streaming
        pb = ExitStack()
        pXg = pb.enter_context(tc.tile_pool(name="pXg", bufs=2))
        pQg = pb.enter_context(tc.tile_pool(name="pQg", bufs=2))
        pPr = pb.enter_context(tc.tile_pool(name="pPr", bufs=2))
        pAcc = pb.enter_context(tc.tile_pool(name="pAcc", bufs=2))
        psm = pb.enter_context(tc.tile_pool(name="psm", bufs=3))

        NIX = P * MC
        for k in range(nchunk):
            r = k // chunks_per_r
            isl = slice(k * 8 * MC, (k + 1) * 8 * MC)
            xit = psm.tile([P, 8 * MC], i16, tag="xit")
            nc.sync.dma_start(xit[:], xid[:, isl])
            qit = psm.tile([P, 8 * MC], i16, tag="qit")
            nc.sync.dma_start(qit[:], qid[:, isl])
            sit = psm.tile([P, 8 * MC], i16, tag="sit")
            nc.sync.dma_start(sit[:], sid[:, isl])

            Xg = pXg.tile([P, MC, CD], f32, tag="Xg")
            Qg = pQg.tile([P, MC, CD], f32, tag="Qg")
            cw = MC // NSUB
            iw = 8 * MC // NSUB
            for s in range(NSUB):
                nc.gpsimd.dma_gather(
                    out_ap=Xg[:, s * cw:(s + 1) * cw, :],
                    in_ap=xtab[r * QSZ:(r + 1) * QSZ, :],
                    idxs_ap=xit[:, s * iw:(s + 1) * iw],
                    num_idxs=NIX // NSUB, num_idxs_reg=NIX // NSUB,
                    elem_size=CD)
                nc.gpsimd.dma_gather(
                    out_ap=Qg[:, s * cw:(s + 1) * cw, :], in_ap=qtab[:],
                    idxs_ap=qit[:, s * iw:(s + 1) * iw],
                    num_idxs=NIX // NSUB, num_idxs_reg=NIX // NSUB,
                    elem_size=CD)

            Pr = pPr.tile([P, MC, CD], f32, tag="Pr")
            nc.vector.tensor_tensor(out=Pr[:], in0=Xg[:], in1=Qg[:],
                                    op=mybir.AluOpType.mult)
            S = psm.tile([P, MC], f32, tag="S")
            nc.vector.tensor_reduce(out=S[:], in_=Pr[:],
                                    axis=mybir.AxisListType.X,
                                    op=mybir.AluOpType.add)
            W = psm.tile([P, MC], f32, tag="W")
            nc.scalar.activation(W[:], S[:], mybir.ActivationFunctionType.Exp)

            Acc = pAcc.tile([P, MC, Cp], f32, tag="Acc")
            nc.gpsimd.tensor_tensor(
                out=Acc[:, :, :CD], in0=Xg[:],
                in1=W[:].unsqueeze(-1).to_broadcast([P, MC, CD]),
                op=mybir.AluOpType.mult)
            nc.vector.tensor_copy(Acc[:, :, CD], W[:])

            for s in range(NSUB):
                nc.gpsimd.dma_scatter_add(
                    out_ap=aggd[:, :Cp], in_ap=Acc[:, s * cw:(s + 1) * cw, :],
                    idxs_ap=sit[:, s * iw:(s + 1) * iw],
                    num_idxs=NIX // NSUB, num_idxs_reg=NIX // NSUB,
                    elem_size=Cp, elem_step=2 * CD)
        pb.close()

        # phase C: normalize + output matmuls
        pc_in = ctx.enter_context(tc.tile_pool(name="pc_in", bufs=3))
        pc_ps = ctx.enter_context(tc.tile_pool(name="pc_ps", bufs=2, space="PSUM"))
        pc_o = ctx.enter_context(tc.tile_pool(name="pc_o", bufs=3))
        for t in range(J):
            ag = pc_in.tile([P, Cp], f32, tag="ag")
            nc.sync.dma_start(ag[:], aggd[t * P:(t + 1) * P, :Cp])
            rc = pc_in.tile([P, 1], f32, tag="rc")
            nc.vector.reciprocal(rc[:], ag[:, CD:CD + 1])
            an = pc_in.tile([P, CD], f32, tag="an")
            nc.vector.tensor_scalar_mul(an[:], ag[:, :CD], rc[:])
            pst = pc_ps.tile([CD, P], f32, tag="pst")
            nc.tensor.transpose(out=pst[:], in_=an[:], identity=ident[:])
            ant = pc_in.tile([CD, P], f32, tag="ant")
            nc.vector.tensor_copy(ant[:], pst[:])
            xt2 = pc_in.tile([CD + 1, P], f32, tag="xt2")
            nc.sync.dma_start(xt2[:], xpt[:, t * P:(t + 1) * P])
            pso = pc_ps.tile([P, Cout], f32, tag="pso")
            nc.tensor.matmul(pso[:], lhsT=ant[:], rhs=Wvsb[:], start=True,
                             stop=False, skip_group_check=True)
            nc.tensor.matmul(pso[:], lhsT=xt2[:], rhs=WsAsb[:], start=False,
                             stop=True, skip_group_check=True)
            ot2 = pc_o.tile([P, Cout], f32, tag="ot2")
            fn = (mybir.ActivationFunctionType.Relu if relu
                  else mybir.ActivationFunctionType.Copy)
            nc.scalar.activation(ot2[:], pso[:], fn)
            nc.sync.dma_start(out[t * P:(t + 1) * P, :], ot2[:])

    nc.compile()
    return nc


def _layer_launch(nc, plan, xfull, A_aug, Wv, Ws_aug, sim=False):
    Cin = xfull.shape[1]
    xpad = xfull
    if Cin < CD:
        xpad = np.zeros((N_NODES, CD), np.float32)
        xpad[:, :Cin] = xfull
    in_maps = []
    for c in range(NCORES):
        pc = plan["cores"][c]
        lo = c * NL
        xperm = np.zeros((NJ, CD), np.float32)
        xperm[:NL] = xpad[lo:lo + NL]
        xpt = np.concatenate([xperm.T, np.ones((1, NJ), np.float32)], axis=0)
        in_maps.append({
            "xtab": np.ascontiguousarray(xpad, np.float32),
            "xpt": np.ascontiguousarray(xpt),
            "xi": pc["xi"], "qi": pc["qi"], "si": pc["si"],
            "Aaug": A_aug, "Wv": Wv, "WsA": Ws_aug,
        })

    if sim:
        from concourse.bass_interp import CoreSim
        results = []
        for c in range(NCORES if sim == "all" else 1):
            s = CoreSim(nc, trace=False, require_finite=False, require_nnan=False)
            for k2, v in in_maps[c].items():
                s.tensor(k2)[:] = v
            s.simulate()
            results.append({"out": np.array(s.tensor("out"))})
        return results, None

    from concourse import bass_utils
    trace = bool(int(os.environ.get("GNN_TRACE", "0")))
    br = bass_utils.run_bass_kernel_spmd(
        nc, in_maps, core_ids=list(range(NCORES)), trace=trace)
    return br.results, br


def kernel(**inputs):
    x = np.ascontiguousarray(np.asarray(inputs["x"], np.float32))
    edge_index = np.asarray(inputs["edge_index"])
    plan = _build_plan(edge_index)
    M = plan["M"]

    cfgs = [(8, 64, True), (64, 64, True), (64, 112, False)]
    prog_cache = {}
    sim = os.environ.get("GNN_SIM", "")
    total_ns = 0
    have_ns = True
    h = x
    for li, (Cin, Cout, relu) in enumerate(cfgs):
        pk = (Cout, relu)
        if pk not in prog_cache:
            prog_cache[pk] = _build_layer_program(Cout, M, relu)
        A_aug, Wv, Ws_aug = _fold_weights(inputs, li)
        results, br = _layer_launch(prog_cache[pk], plan, h, A_aug, Wv, Ws_aug,
                                    sim=sim)
        hn = np.zeros((N_NODES, Cout), np.float32)
        for c in range(len(results)):
            hn[c * NL:(c + 1) * NL] = results[c]["out"][:NL]
        h = hn
        if br is not None and br.exec_time_ns:
            total_ns += br.exec_time_ns
        else:
            have_ns = False

    if have_ns and total_ns:
        kernel.last_exec_ns = total_ns
    return h


kernel.last_exec_ns = None

